# revision 7
# baseline (speedup 1.0000x reference)
"""Distributed Bass kernel: causal multi-head attention with RoPE.

Full op:  x[2,2048,2048] -> attention(16 heads, RoPE, causal) @ wo.T
Sharding: core = b*4 + j  (b in {0,1} batch, j in {0..3} head-group)
  - core owns batch b, heads 4j..4j+3 (512 of the 2048 hidden dims)
  - QKV projections computed locally from x[b].T (replicated per group)
  - attention computed in "transposed scores" layout (scoresT[sk, sq]) so
    softmax sums are PE ones-matmuls and no probability transposes needed
  - AllGather (groups [0..3], [4..7]) of yT = attn-out.T across the group
  - each core computes output COLUMNS j*512..(j+1)*512 = Y @ wo[jslice].T
Compute dtype bf16 (f32 accumulation in PSUM); inputs converted on host.
"""

import math
import os
import sys

for _p in ("/opt/trn_rl_repo",):
    if _p not in sys.path:
        sys.path.insert(0, _p)

import ml_dtypes
import numpy as np

import concourse.bass as bass  # noqa: F401
import concourse.mybir as mybir
import concourse.tile as tile
from concourse import bacc
from concourse.bass_utils import run_bass_kernel_spmd

BF16 = mybir.dt.bfloat16
F32 = mybir.dt.float32
NPBF16 = ml_dtypes.bfloat16

B, S, D = 2, 2048, 2048
H, HD = 16, 128
BASE = 10000
NCORES = 8
GROUPS = [[0, 1, 2, 3], [4, 5, 6, 7]]
HPC = 4            # heads per core
DPC = HPC * HD     # 512 hidden dims per core
KC = D // 128      # 16 contraction chunks
NS = S // 512      # 4 seq slices of 512
SCALE = 1.0 / math.sqrt(HD)
NEG = -30000.0

_CACHE = {}

LAST_EXEC_NS = None
LAST_TRACE = None


def _install_ntff_hook():
    """The image's antenv lacks axon_hooks; bass_utils hard-imports it when
    trace=True. Register the boot module's ctypes hook under that name."""
    try:
        import antenv.axon_hooks  # noqa: F401
        return True
    except ImportError:
        pass
    try:
        import types

        import antenv
        from trn_agent_boot.trn_boot import _ntff_profile_via_ctypes

        mod = types.ModuleType("antenv.axon_hooks")
        _hook = [None]
        mod.set_axon_ntff_profile_hook = lambda h: _hook.__setitem__(0, h)
        mod.get_axon_ntff_profile_hook = lambda: _hook[0]
        sys.modules["antenv.axon_hooks"] = mod
        antenv.axon_hooks = mod
        mod.set_axon_ntff_profile_hook(
            _ntff_profile_via_ctypes("/opt/axon/libaxon_pjrt.so")
        )
        return True
    except Exception:
        return False


def _build():
    nc = bacc.Bacc(None, target_bir_lowering=False, num_devices=NCORES)

    xT = nc.declare_dram_parameter("xT", [D, S], BF16, isOutput=False)
    wqT = nc.declare_dram_parameter("wqT", [D, DPC], BF16, isOutput=False)
    wkT = nc.declare_dram_parameter("wkT", [D, DPC], BF16, isOutput=False)
    wvT = nc.declare_dram_parameter("wvT", [D, DPC], BF16, isOutput=False)
    woT = nc.declare_dram_parameter("woT", [D, DPC], BF16, isOutput=False)
    cosE = nc.declare_dram_parameter("cosE", [HD, S], BF16, isOutput=False)
    sinE = nc.declare_dram_parameter("sinE", [HD, S], BF16, isOutput=False)
    # aux: rows 0..127 stacked [P | btri | ident], plus ones columns
    pswap = nc.declare_dram_parameter("pswap", [128, 128], BF16, isOutput=False)
    btri = nc.declare_dram_parameter("btri", [128, 128], BF16, isOutput=False)
    ident = nc.declare_dram_parameter("ident", [128, 128], BF16, isOutput=False)
    ones2 = nc.declare_dram_parameter("ones2", [128, 128], BF16, isOutput=False)
    out = nc.declare_dram_parameter("out", [S, DPC], F32, isOutput=True)

    with tile.TileContext(nc) as tc:
        with (
            tc.tile_pool(name="consts", bufs=1) as cpool,
            tc.tile_pool(name="qkv", bufs=1) as qkvp,
            tc.tile_pool(name="dram", bufs=1, space="DRAM") as dpool,
            tc.tile_pool(name="ytout", bufs=4) as ytp,
        ):
            cos_t = cpool.tile([HD, S], BF16, tag="cos", name="cos")
            nc.sync.dma_start(out=cos_t[:], in_=cosE[:, :])
            sin_t = cpool.tile([HD, S], BF16, tag="sin", name="sin")
            nc.sync.dma_start(out=sin_t[:], in_=sinE[:, :])
            p_t = cpool.tile([128, 128], BF16, tag="pswap", name="pswap")
            nc.sync.dma_start(out=p_t[:], in_=pswap[:, :])
            btri_t = cpool.tile([128, 128], BF16, tag="btri", name="btri")
            nc.sync.dma_start(out=btri_t[:], in_=btri[:, :])
            id_t = cpool.tile([128, 128], BF16, tag="ident", name="ident")
            nc.sync.dma_start(out=id_t[:], in_=ident[:, :])
            ones_t = cpool.tile([128, 128], BF16, tag="ones2", name="ones2")
            nc.sync.dma_start(out=ones_t[:], in_=ones2[:, :])

            wo_t = []
            for k in range(KC):
                t = cpool.tile([128, DPC], BF16, tag=f"wo{k}", name=f"wo{k}")
                nc.sync.dma_start(out=t[:], in_=woT[k * 128:(k + 1) * 128, :])
                wo_t.append(t)

            qT = [qkvp.tile([HD, S], BF16, tag=f"q{h}", name=f"q{h}") for h in range(HPC)]
            kT = [qkvp.tile([HD, S], BF16, tag=f"k{h}", name=f"k{h}") for h in range(HPC)]
            vv = [qkvp.tile([128, DPC], BF16, tag=f"v{m}", name=f"v{m}") for m in range(KC)]

            bounce_in = dpool.tile([DPC, S], BF16, tag="bin", name="bin")
            bounce_out = dpool.tile([D, S], BF16, tag="bout", name="bout")

            # ---------------- phase 1: projections + RoPE ----------------
            with (
                tc.tile_pool(name="xt", bufs=1) as xtp,
                tc.tile_pool(name="wgt", bufs=1) as wp,
                tc.tile_pool(name="ps1", bufs=2, space="PSUM") as ps1,
                tc.tile_pool(name="psw", bufs=2, space="PSUM") as psw,
                tc.tile_pool(name="rtmp", bufs=4) as rtmp,
            ):
                xt = []
                for k in range(KC):
                    t = xtp.tile([128, S], BF16, tag=f"x{k}", name=f"x{k}")
                    nc.sync.dma_start(out=t[:], in_=xT[k * 128:(k + 1) * 128, :])
                    xt.append(t)
                w_t = {}
                for nm, src in (("q", wqT), ("k", wkT), ("v", wvT)):
                    lst = []
                    for k in range(KC):
                        t = wp.tile([128, DPC], BF16, tag=f"w{nm}{k}", name=f"w{nm}{k}")
                        nc.sync.dma_start(out=t[:], in_=src[k * 128:(k + 1) * 128, :])
                        lst.append(t)
                    w_t[nm] = lst

                # v projection: vv[m][s-chunk, 512 dims]
                for m in range(KC):
                    pv = ps1.tile([128, DPC], F32, tag="pv", name="pv")
                    for k in range(KC):
                        nc.tensor.matmul(
                            pv[:], xt[k][:, m * 128:(m + 1) * 128], w_t["v"][k][:],
                            start=(k == 0), stop=(k == KC - 1),
                        )
                    nc.vector.tensor_copy(vv[m][:], pv[:])

                # q/k projections (transposed layout) + RoPE
                for nm, dst in (("q", qT), ("k", kT)):
                    for h in range(HPC):
                        for n in range(NS):
                            pq = ps1.tile([128, 512], F32, tag="pq", name="pq")
                            for k in range(KC):
                                nc.tensor.matmul(
                                    pq[:],
                                    w_t[nm][k][:, h * 128:(h + 1) * 128],
                                    xt[k][:, n * 512:(n + 1) * 512],
                                    start=(k == 0), stop=(k == KC - 1),
                                )
                            raw = rtmp.tile([128, 512], BF16, tag="raw", name="raw")
                            nc.vector.tensor_copy(raw[:], pq[:])
                            sw = psw.tile([128, 512], F32, tag="sw", name="sw")
                            nc.tensor.matmul(sw[:], p_t[:], raw[:], start=True, stop=True)
                            t1 = rtmp.tile([128, 512], BF16, tag="t1", name="t1")
                            nc.vector.tensor_tensor(
                                t1[:], raw[:], cos_t[:, n * 512:(n + 1) * 512],
                                mybir.AluOpType.mult,
                            )
                            t2 = rtmp.tile([128, 512], BF16, tag="t2", name="t2")
                            nc.vector.tensor_tensor(
                                t2[:], sw[:], sin_t[:, n * 512:(n + 1) * 512],
                                mybir.AluOpType.mult,
                            )
                            nc.vector.tensor_tensor(
                                dst[h][:, n * 512:(n + 1) * 512], t1[:], t2[:],
                                mybir.AluOpType.add,
                            )

            # ---------------- phase 2: attention ----------------
            with (
                tc.tile_pool(name="pssc", bufs=3, space="PSUM") as pssc,
                tc.tile_pool(name="psyt", bufs=2, space="PSUM") as psyt,
                tc.tile_pool(name="pssum", bufs=2, space="PSUM") as pssum,
                tc.tile_pool(name="probs", bufs=8) as prp,
                tc.tile_pool(name="fin", bufs=4) as finp,
            ):
                for h in range(HPC):
                    for slc in range(NS):
                        sq0 = slc * 512
                        pyt = psyt.tile([128, 512], F32, tag="pyt", name="pyt")
                        psm = pssum.tile([1, 512], F32, tag="psm", name="psm")
                        nchunks = slc * 4 + 4
                        for kk in range(nchunks):
                            diag = kk >= slc * 4
                            dlt = (kk - slc * 4) * 128 if diag else 0
                            psc = pssc.tile([128, 512], F32, tag="psc", name="psc")
                            nc.tensor.matmul(
                                psc[:, dlt:512],
                                kT[h][:, kk * 128:(kk + 1) * 128],
                                qT[h][:, sq0 + dlt:sq0 + 512],
                                start=True, stop=not diag,
                            )
                            if diag:
                                nc.tensor.matmul(
                                    psc[:, dlt:dlt + 128], id_t[:], btri_t[:],
                                    start=False, stop=True,
                                )
                            pb = prp.tile([128, 512], BF16, tag="pb", name="pb")
                            nc.scalar.activation(
                                pb[:, dlt:512], psc[:, dlt:512],
                                mybir.ActivationFunctionType.Exp, scale=SCALE,
                            )
                            nc.tensor.matmul(
                                psm[0:1, dlt:512], ones_t[:, 0:1], pb[:, dlt:512],
                                start=(kk == 0), stop=(kk == nchunks - 1),
                            )
                            nc.tensor.matmul(
                                pyt[:, dlt:512],
                                vv[kk][:, h * 128:(h + 1) * 128], pb[:, dlt:512],
                                start=(kk == 0), stop=(kk == nchunks - 1),
                            )
                        rc = finp.tile([1, 512], F32, tag="rc", name="rc")
                        nc.vector.reciprocal(rc[:], psm[:])
                        rcb = finp.tile([1, 512], BF16, tag="rcb", name="rcb")
                        nc.vector.tensor_copy(rcb[:], rc[:])
                        prb = pssc.tile([128, 512], F32, tag="psc", name="psc")
                        nc.tensor.matmul(
                            prb[:], ones_t[0:1, :], rcb[0:1, :], start=True, stop=True,
                        )
                        rbs = finp.tile([128, 512], F32, tag="rbs", name="rbs")
                        nc.vector.tensor_copy(rbs[:], prb[:])
                        yt = ytp.tile([128, 512], BF16, tag="yt", name="yt")
                        nc.vector.tensor_tensor(
                            yt[:], pyt[:], rbs[:], mybir.AluOpType.mult,
                        )
                        nc.sync.dma_start(
                            out=bounce_in[h * 128:(h + 1) * 128, sq0:sq0 + 512],
                            in_=yt[:],
                        )

                nc.gpsimd.collective_compute(
                    "AllGather",
                    mybir.AluOpType.bypass,
                    replica_groups=GROUPS,
                    ins=[bounce_in[:].opt()],
                    outs=[bounce_out[:].opt()],
                )

            # ---------------- phase 3: output projection ----------------
            with (
                tc.tile_pool(name="yts", bufs=1) as ytsp,
                tc.tile_pool(name="pso", bufs=4, space="PSUM") as pso,
                tc.tile_pool(name="ost", bufs=4) as ostp,
            ):
                ytk = []
                for k in range(KC):
                    t = ytsp.tile([128, S], BF16, tag=f"yt{k}", name=f"yt{k}")
                    nc.sync.dma_start(out=t[:], in_=bounce_out[k * 128:(k + 1) * 128, :])
                    ytk.append(t)
                for m in range(KC):
                    po = pso.tile([128, DPC], F32, tag="po", name="po")
                    for k in range(KC):
                        nc.tensor.matmul(
                            po[:], ytk[k][:, m * 128:(m + 1) * 128], wo_t[k][:],
                            start=(k == 0), stop=(k == KC - 1),
                        )
                    ot = ostp.tile([128, DPC], F32, tag="ot", name="ot")
                    nc.vector.tensor_copy(ot[:], po[:])
                    nc.sync.dma_start(out=out[m * 128:(m + 1) * 128, :], in_=ot[:])

    nc.finalize()
    return nc


def _host_consts():
    theta = 1.0 / (BASE ** (np.arange(0, HD, 2, dtype=np.float64)[: HD // 2] / HD))
    idx = np.arange(S, dtype=np.float64)[:, None] * theta[None, :]  # [S, 64]
    cos = np.cos(idx).astype(np.float32)
    sin = np.sin(idx).astype(np.float32)
    cosE = np.repeat(cos.T, 2, axis=0)          # [128, S]
    sinE = np.repeat(sin.T, 2, axis=0)
    sinE[0::2, :] *= -1.0                        # even rows: -sin
    P = np.zeros((128, 128), np.float32)
    P[np.arange(128), np.arange(128) ^ 1] = 1.0
    btri = np.where(
        np.arange(128)[:, None] > np.arange(128)[None, :], NEG, 0.0
    ).astype(np.float32)
    ident = np.eye(128, dtype=np.float32)
    ones2 = np.ones((128, 128), np.float32)
    return {
        "cosE": cosE.astype(NPBF16),
        "sinE": sinE.astype(NPBF16),
        "pswap": P.astype(NPBF16),
        "btri": btri.astype(NPBF16),
        "ident": ident.astype(NPBF16),
        "ones2": ones2.astype(NPBF16),
    }


def kernel(x, mask, wq, wk, wv, wo):
    global LAST_EXEC_NS, LAST_TRACE
    x = np.asarray(x, dtype=np.float32)
    wq = np.asarray(wq, dtype=np.float32)
    wk = np.asarray(wk, dtype=np.float32)
    wv = np.asarray(wv, dtype=np.float32)
    wo = np.asarray(wo, dtype=np.float32)

    consts = _host_consts()
    in_maps = []
    for core in range(NCORES):
        b, j = core // 4, core % 4
        sl = slice(j * DPC, (j + 1) * DPC)
        m = {
            "xT": np.ascontiguousarray(x[b].T).astype(NPBF16),
            "wqT": np.ascontiguousarray(wq[sl, :].T).astype(NPBF16),
            "wkT": np.ascontiguousarray(wk[sl, :].T).astype(NPBF16),
            "wvT": np.ascontiguousarray(wv[sl, :].T).astype(NPBF16),
            "woT": np.ascontiguousarray(wo[sl, :].T).astype(NPBF16),
        }
        m.update(consts)
        in_maps.append(m)

    if "nc" not in _CACHE:
        _CACHE["nc"] = _build()
    nc = _CACHE["nc"]

    trace = os.environ.get("KERNEL_TRACE", "0") == "1"
    if trace:
        trace = _install_ntff_hook()
    res = run_bass_kernel_spmd(
        nc, in_maps, core_ids=list(range(NCORES)), trace=trace,
    )
    LAST_EXEC_NS = getattr(res, "exec_time_ns", None)
    LAST_TRACE = getattr(res, "instructions_and_trace", None)

    out = np.empty((B, S, D), np.float32)
    for core in range(NCORES):
        b, j = core // 4, core % 4
        out[b, :, j * DPC:(j + 1) * DPC] = np.asarray(
            res.results[core]["out"], dtype=np.float32
        )
    return out
